# revision 11
# baseline (speedup 1.0000x reference)
"""Trainium2 Bass kernel for nn_KASR_66005057405539 (KGAT-style recommender).

Strategy (8 NeuronCores):
- Host resolves the 2-hop KG index chains and materializes per-token bf16
  embedding streams in fused tile layout (pure indexing/layout work).
- Launch A (batch-sharded, 32 batches/core): 2 attention-aggregation hops
  processed as 4-tile supergroups (bf16 DVE 2x ops, fold-tree reductions,
  one wide Wt matmul per group), 50-step GRU, attention pooling
  -> ghtT [128d, 32b] per core.
- Launch B (vocab-sharded, 12500 items/core): logits = relu(ght @ item_emb.T)
  in bf16.
All floating-point math runs on device.
"""

import sys

sys.path.insert(0, "/root/problem")
import ml_dtypes
import numpy as np

import concourse.bacc as bacc
import concourse.mybir as mybir
import concourse.tile as tile
from concourse.bass_utils import run_bass_kernel_spmd
from concourse.masks import make_identity

F32 = mybir.dt.float32
BF16 = mybir.dt.bfloat16
AF = mybir.ActivationFunctionType
ALU = mybir.AluOpType

B, S, NB, D = 256, 50, 8, 128
N_ITEMS, N_RELS = 100000, 200
NCORE = 8
BC = B // NCORE  # 32 batches per core
F1 = BC * S  # 1600 hop-1 tokens per core
T1 = 13  # hop-1 tiles (1600 -> pad 1664)
F1P = T1 * 128  # 1664
T0 = T1 * 8  # 104 hop-0 tiles (13312 tokens)
VS = N_ITEMS // NCORE  # 12500 vocab per core
VSP = 25 * 512  # 12800 padded
FW = 2176  # fused stream row: self(128) | rel(1024) | nb(1024)
FW1 = 1152  # hop-1 stream row: self(128) | rel(1024)

_CACHE = {}
PROFILE = {}


def _run(nc, in_maps, label):
    import os

    trace = os.environ.get("KASR_PROFILE") == "1"
    if trace:
        try:
            r = run_bass_kernel_spmd(nc, in_maps, list(range(NCORE)), trace=True)
            PROFILE[label] = r.exec_time_ns
            return r
        except Exception:
            PROFILE[label] = None
    return run_bass_kernel_spmd(nc, in_maps, list(range(NCORE)))


def _hop_group(nc, pools, G, st3, nbv, wa_b, wt_t, bt_c, identb, out_fn):
    """One supergroup of G 128-token tiles (token-major, bf16).

    st3: stream view [128, G, FW-or-FW1] (self | rel [| nb]).
    nbv: neighbor view [128, G, 1024] (from stream or from neib0).
    out_fn(ps): consume the D-major Wt output PSUM [128, G*128].
    """
    stream, work, psum = pools
    selfv = st3[:, :, 0:D]
    relv = st3[:, :, D:D + NB * D]

    sfw = work.tile([128, G * D], BF16, tag="sfw")
    sfw3 = sfw[:].rearrange("p (u d) -> p u d", u=G)
    wab_b = wa_b[:].unsqueeze(1).broadcast_to([128, G, D])
    nc.vector.tensor_tensor(out=sfw3, in0=selfv, in1=wab_b, op=ALU.mult)

    prod = work.tile([128, G * NB * D], BF16, tag="prod")
    prod3 = prod[:].rearrange("p (u f) -> p u f", u=G)
    nc.vector.tensor_tensor(out=prod3, in0=relv, in1=nbv, op=ALU.mult)

    # p2 = prod * self*Wa (broadcast over n)
    p2 = work.tile([128, G * NB * D], BF16, tag="p2")
    p2v = p2[:].rearrange("p (u n d) -> p u n d", u=G, n=NB)
    prodv = prod[:].rearrange("p (u n d) -> p u n d", u=G, n=NB)
    sfw_b = sfw3.unsqueeze(2).broadcast_to([128, G, NB, D])
    nc.vector.tensor_tensor(out=p2v, in0=prodv, in1=sfw_b, op=ALU.mult)

    # att = sum_d p2 : fold d 128->64->32, then reduce
    f1 = work.tile([128, G * NB * 64], BF16, tag="f1")
    f1v = f1[:].rearrange("p (g s) -> p g s", s=64)
    nc.vector.tensor_tensor(out=f1v, in0=p2v[:, :, :, 0:64].rearrange(
        "p u n s -> p (u n) s"), in1=p2v[:, :, :, 64:128].rearrange(
        "p u n s -> p (u n) s"), op=ALU.add)
    f2 = work.tile([128, G * NB * 32], BF16, tag="f2")
    f2v = f2[:].rearrange("p (g s) -> p g s", s=32)
    nc.vector.tensor_tensor(out=f2v, in0=f1v[:, :, 0:32], in1=f1v[:, :, 32:64],
                            op=ALU.add)
    att = work.tile([128, G * NB], F32, tag="att")
    nc.vector.tensor_reduce(out=att[:], in_=f2v, axis=mybir.AxisListType.X,
                            op=ALU.add)

    # softmax over n (groups of 8): exp + per-tile row-sum accumulators
    e = work.tile([128, G * NB], BF16, tag="e")
    se = work.tile([128, G], F32, tag="se")
    for u in range(G):
        nc.scalar.activation(out=e[:, u * NB:(u + 1) * NB],
                             in_=att[:, u * NB:(u + 1) * NB], func=AF.Exp,
                             accum_out=se[:, u:u + 1])
    rec = work.tile([128, G], F32, tag="rec")
    nc.vector.reciprocal(out=rec[:], in_=se[:])
    alpha = work.tile([128, G * NB], BF16, tag="alpha")
    av = alpha[:].rearrange("p (u n) -> p u n", u=G)
    ev = e[:].rearrange("p (u n) -> p u n", u=G)
    rec_b = rec[:].unsqueeze(2).broadcast_to([128, G, NB])
    nc.vector.tensor_tensor(out=av, in0=ev, in1=rec_b, op=ALU.mult)
    # pair-expand alpha along d so the weighting multiply keeps a packed
    # 2-byte last dim (DVE 2x)
    alphaX = work.tile([128, G * NB * 2], BF16, tag="alphaX")
    aXv = alphaX[:].rearrange("p (g two) -> p g two", two=2)
    a_b = alpha[:].unsqueeze(2).broadcast_to([128, G * NB, 2])
    nc.vector.tensor_scalar_mul(out=aXv, in0=a_b, scalar1=1.0)

    wnb = work.tile([128, G * NB * D], BF16, tag="wnb")
    wnbv = wnb[:].rearrange("p (u g s two) -> p u g s two", u=G, g=NB, two=2)
    nb4 = nbv.rearrange("p u (g s two) -> p u g s two", g=NB, two=2)
    aX_b = alphaX[:].rearrange("p (u g two) -> p u g two", u=G, two=2).unsqueeze(
        3).broadcast_to([128, G, NB, 64, 2])
    nc.vector.tensor_tensor(out=wnbv, in0=nb4, in1=aX_b, op=ALU.mult)

    # agg = sum_n wnb : fold n 8->4->2->1
    w4 = wnb[:].rearrange("p (u n d) -> p u n d", u=G, n=NB)
    t1 = work.tile([128, G * 4 * D], BF16, tag="t1")
    t1v = t1[:].rearrange("p (u n d) -> p u n d", u=G, n=4)
    nc.vector.tensor_tensor(out=t1v, in0=w4[:, :, 0:4, :], in1=w4[:, :, 4:8, :],
                            op=ALU.add)
    t2 = work.tile([128, G * 2 * D], BF16, tag="t2")
    t2v = t2[:].rearrange("p (u n d) -> p u n d", u=G, n=2)
    nc.vector.tensor_tensor(out=t2v, in0=t1v[:, :, 0:2, :], in1=t1v[:, :, 2:4, :],
                            op=ALU.add)
    ags = work.tile([128, G * D], BF16, tag="ags")
    agsv = ags[:].rearrange("p (u d) -> p u d", u=G)
    # fold last pair and add self in two ops: agg2 = n0 + n1 ; ags = agg2 + self
    agg = work.tile([128, G * D], BF16, tag="agg")
    aggv = agg[:].rearrange("p (u d) -> p u d", u=G)
    nc.vector.tensor_tensor(out=aggv, in0=t2v[:, :, 0, :], in1=t2v[:, :, 1, :],
                            op=ALU.add)
    nc.vector.tensor_tensor(out=agsv, in0=aggv, in1=selfv, op=ALU.add)

    # (self+agg) @ Wt : transpose each tile to D-major, one wide matmul
    tps = work.tile([128, G * D], BF16, tag="tps")
    for u in range(G):
        tp = psum.tile([128, D], BF16, tag="tp")
        nc.tensor.transpose(out=tp[:], in_=ags[:, u * D:(u + 1) * D],
                            identity=identb[:])
        nc.scalar.copy(out=tps[:, u * D:(u + 1) * D], in_=tp[:])
    mm = psum.tile([128, G * D], F32, tag="mm")
    nc.tensor.matmul(out=mm[:], lhsT=wt_t[:], rhs=tps[:], start=True, stop=True)
    out_fn(mm)


def build_launch_a():
    nc = bacc.Bacc(None)
    dp = nc.declare_dram_parameter
    stream0 = dp("stream0", [128, T0, FW], BF16, isOutput=False)
    stream1 = dp("stream1", [128, T1, FW1], BF16, isOutput=False)
    wa_p = dp("wa_b", [128, D], BF16, isOutput=False)
    wt_p = dp("wt", [D, D], BF16, isOutput=False)
    bt_p = dp("bt", [128, 1], F32, isOutput=False)
    wih_p = dp("wih", [D, 3 * D], BF16, isOutput=False)
    whh_p = dp("whh", [D, 3 * D], BF16, isOutput=False)
    biasr_p = dp("biasr", [128, 1], F32, isOutput=False)
    biasz_p = dp("biasz", [128, 1], F32, isOutput=False)
    nbiasz_p = dp("nbiasz", [128, 1], F32, isOutput=False)
    biasin_p = dp("biasin", [128, 1], F32, isOutput=False)
    biashn_p = dp("biashn", [128, 1], F32, isOutput=False)
    w1_p = dp("w1", [D, D], BF16, isOutput=False)
    b1_p = dp("b1", [128, 1], F32, isOutput=False)
    w2_p = dp("w2", [D, D], BF16, isOutput=False)
    b2_p = dp("b2", [128, 1], F32, isOutput=False)
    w3_p = dp("w3", [128, 1], F32, isOutput=False)
    wtr0_p = dp("wtr0", [D, D], BF16, isOutput=False)
    wtr1_p = dp("wtr1", [D, D], BF16, isOutput=False)
    btr_p = dp("btr", [128, 1], F32, isOutput=False)
    bmask_p = dp("bmask", [128, T1 * BC], BF16, isOutput=False)
    ght_o = dp("ghtT", [128, BC], F32, isOutput=True)

    with tile.TileContext(nc) as tc:
        with (
            tc.tile_pool(name="const", bufs=1) as const,
            tc.tile_pool(name="stream", bufs=3) as stream,
            tc.tile_pool(name="work", bufs=2) as work,
        ):
            ident = const.tile([128, 128], F32)
            make_identity(nc, ident[:])
            identb = const.tile([128, 128], BF16)
            nc.gpsimd.memset(identb[:], 0.0)
            nc.vector.tensor_scalar_add(out=identb[:], in0=ident[:], scalar1=0.0)

            def load(name, p, shape, dt):
                t = const.tile(shape, dt, tag=name)
                nc.sync.dma_start(out=t[:], in_=p[:])
                return t

            wa_b = load("wa_b", wa_p, [128, D], BF16)
            wt_t = load("wt", wt_p, [D, D], BF16)
            bt_c = load("bt", bt_p, [128, 1], F32)
            wih_t = load("wih", wih_p, [D, 3 * D], BF16)
            whh_t = load("whh", whh_p, [D, 3 * D], BF16)
            small = {}
            for nm, p in (("biasr", biasr_p), ("biasz", biasz_p),
                          ("nbiasz", nbiasz_p), ("biasin", biasin_p),
                          ("biashn", biashn_p), ("b1", b1_p), ("b2", b2_p),
                          ("w3", w3_p), ("btr", btr_p)):
                small[nm] = load(nm, p, [128, 1], F32)
            w1_t = load("w1", w1_p, [D, D], BF16)
            w2_t = load("w2", w2_p, [D, D], BF16)
            wtr0_t = load("wtr0", wtr0_p, [D, D], BF16)
            wtr1_t = load("wtr1", wtr1_p, [D, D], BF16)
            bm_t = load("bmask", bmask_p, [128, T1 * BC], BF16)
            ones_c = const.tile([128, 1], BF16)
            nc.gpsimd.memset(ones_c[:], 1.0)
            zero32 = const.tile([128, BC], BF16)
            nc.gpsimd.memset(zero32[:], 0.0)

            neib0 = const.tile([128, T0 * D], BF16)  # hop-0 out, token-major
            xt = const.tile([128, F1P], BF16)  # hop-1 out, D-major
            outt = const.tile([128, F1P], BF16)  # GRU out, D-major
            nc.gpsimd.memset(outt[:, F1:], 0.0)
            w3s = const.tile([128, F1P], BF16)
            albuf = const.tile([128, T1], F32)

            hpsum = tc.tile_pool(name="hpsum", bufs=1, space="PSUM")
            gpsum = tc.tile_pool(name="gpsum", bufs=1, space="PSUM")
            psum = hpsum.__enter__()
            gps_pool = gpsum.__enter__()
            pools = (stream, work, psum)
            G = 4

            def hop0_group(j):
                st = stream.tile([128, G * FW], BF16, tag="st0")
                st3 = st[:].rearrange("p (u f) -> p u f", u=G)
                nc.sync.dma_start(out=st3, in_=stream0[:, j * G:(j + 1) * G])
                nbv = st3[:, :, D + NB * D:FW]

                def out0(mm, j=j):
                    nbt = work.tile([128, G * D], BF16, tag="nbt")
                    nc.scalar.add(out=nbt[:], in_=mm[:], add=bt_c[:, :1])
                    for u in range(G):
                        tb = psum.tile([128, D], BF16, tag="tb")
                        nc.tensor.transpose(out=tb[:],
                                            in_=nbt[:, u * D:(u + 1) * D],
                                            identity=identb[:])
                        g = (j * G + u) * D
                        nc.scalar.copy(out=neib0[:, g:g + D], in_=tb[:])

                _hop_group(nc, pools, G, st3, nbv, wa_b, wt_t, bt_c,
                           identb, out0)

            def hop1_group(lo, G1):
                st = stream.tile([128, 4 * FW1], BF16, tag="st1")
                st3 = st[:].rearrange("p (u f) -> p u f", u=4)[:, :G1]
                nc.sync.dma_start(out=st3, in_=stream1[:, lo:lo + G1])
                nbv = neib0[:, lo * NB * D:(lo + G1) * NB * D].rearrange(
                    "p (u f) -> p u f", u=G1)

                def out1(mm, lo=lo, G1=G1):
                    nc.scalar.add(out=xt[:, lo * D:(lo + G1) * D],
                                  in_=mm[:], add=bt_c[:, :1])

                _hop_group(nc, pools, G1, st3, nbv, wa_b, wt_t, bt_c,
                           identb, out1)

            def gru_step(t):
                x_t = xt[:, t * BC:(t + 1) * BC]
                h_prev = (outt[:, (t - 1) * BC:t * BC] if t > 0 else zero32[:])
                psr = gps_pool.tile([128, BC], F32, tag="psr")
                nc.tensor.matmul(out=psr[:], lhsT=wih_t[:, 0:D], rhs=x_t,
                                 start=True, stop=False)
                nc.tensor.matmul(out=psr[:], lhsT=whh_t[:, 0:D], rhs=h_prev,
                                 start=False, stop=True)
                r = work.tile([128, BC], BF16, tag="gr")
                nc.scalar.activation(out=r[:], in_=psr[:], func=AF.Sigmoid,
                                     bias=small["biasr"][:, :1])
                psz = gps_pool.tile([128, BC], F32, tag="psz")
                nc.tensor.matmul(out=psz[:], lhsT=wih_t[:, D:2 * D], rhs=x_t,
                                 start=True, stop=False)
                nc.tensor.matmul(out=psz[:], lhsT=whh_t[:, D:2 * D], rhs=h_prev,
                                 start=False, stop=True)
                z = work.tile([128, BC], BF16, tag="gz")
                nc.scalar.activation(out=z[:], in_=psz[:], func=AF.Sigmoid,
                                     bias=small["biasz"][:, :1])
                zm = work.tile([128, BC], BF16, tag="gzm")
                nc.scalar.activation(out=zm[:], in_=psz[:], func=AF.Sigmoid,
                                     scale=-1.0, bias=small["nbiasz"][:, :1])
                zh = work.tile([128, BC], BF16, tag="zh")
                nc.vector.tensor_tensor(out=zh[:], in0=z[:], in1=h_prev,
                                        op=ALU.mult)
                psin = gps_pool.tile([128, BC], F32, tag="psin")
                nc.tensor.matmul(out=psin[:], lhsT=wih_t[:, 2 * D:], rhs=x_t,
                                 start=True, stop=True)
                pshn = gps_pool.tile([128, BC], F32, tag="pshn")
                nc.tensor.matmul(out=pshn[:], lhsT=whh_t[:, 2 * D:], rhs=h_prev,
                                 start=True, stop=True)
                psinb = work.tile([128, BC], BF16, tag="psinb")
                nc.vector.tensor_scalar_add(out=psinb[:], in0=psin[:],
                                            scalar1=small["biasin"][:, :1])
                rhn = work.tile([128, BC], BF16, tag="rhn")
                nc.vector.scalar_tensor_tensor(
                    out=rhn[:], in0=pshn[:], scalar=small["biashn"][:, :1],
                    in1=r[:], op0=ALU.add, op1=ALU.mult)
                pre = work.tile([128, BC], BF16, tag="pre")
                nc.vector.tensor_tensor(out=pre[:], in0=rhn[:], in1=psinb[:],
                                        op=ALU.add)
                n_sb = work.tile([128, BC], BF16, tag="n")
                nc.scalar.activation(out=n_sb[:], in_=pre[:], func=AF.Tanh)
                nzm = work.tile([128, BC], BF16, tag="nzm")
                nc.vector.tensor_tensor(out=nzm[:], in0=n_sb[:], in1=zm[:],
                                        op=ALU.mult)
                nc.vector.tensor_tensor(out=outt[:, t * BC:(t + 1) * BC],
                                        in0=nzm[:], in1=zh[:], op=ALU.add)

            # interleave: hop-0 groups feed hop-1 groups feed GRU steps; GRU
            # steps are spread between hop-0 groups so the recurrence chain's
            # stalls absorb into independent hop work (engine queues are
            # in-order, so coarse blocks would stall the DVE queue)
            done0 = 0
            hop1s = {8: (0, 4), 16: (4, 4), 24: (8, 4), 26: (12, 1)}
            ready_steps = 0
            next_step = 0
            while done0 < 26 or next_step < S:
                if done0 < 26:
                    hop0_group(done0)
                    done0 += 1
                if done0 in hop1s:
                    lo, G1 = hop1s.pop(done0)
                    hop1_group(lo, G1)
                    ready_steps = min(S, (lo + G1) * 4)
                # after the first hop-1 group, drip 2 GRU steps per hop-0 group
                budget = 2 if done0 < 26 else S
                while next_step < ready_steps and budget > 0:
                    gru_step(next_step)
                    next_step += 1
                    budget -= 1
            gpsum.__exit__(None, None, None)
            hpsum.__exit__(None, None, None)

            # ---- attention pooling ----
            ppsum = tc.tile_pool(name="ppsum", bufs=1, space="PSUM")
            psum = ppsum.__enter__()
            loc = work.tile([128, BC], BF16, tag="loc")
            nc.scalar.copy(out=loc[:], in_=outt[:, (S - 1) * BC:S * BC])
            q1ps = psum.tile([128, BC], F32, tag="q1")
            nc.tensor.matmul(out=q1ps[:], lhsT=w1_t[:], rhs=loc[:], start=True,
                             stop=True)
            q1s = work.tile([128, BC], BF16, tag="q1s")
            nc.scalar.add(out=q1s[:], in_=q1ps[:], add=small["b1"][:, :1])
            CH = 10 * BC  # 320 cols = 10 s-steps per chunk (s-major layout)
            for k in range(5):
                ps = psum.tile([128, CH], F32, tag="q2")
                nc.tensor.matmul(out=ps[:], lhsT=w2_t[:],
                                 rhs=outt[:, k * CH:(k + 1) * CH],
                                 start=True, stop=True)
                v = work.tile([128, CH], BF16, tag="v")
                q1b = q1s[:].unsqueeze(1).broadcast_to([128, 10, BC])
                vv = v[:].rearrange("p (s b) -> p s b", b=BC)
                psv = ps[:].rearrange("p (s b) -> p s b", b=BC)
                nc.vector.tensor_tensor(out=vv, in0=psv, in1=q1b, op=ALU.add)
                sg = work.tile([128, CH], BF16, tag="sg")
                nc.scalar.activation(out=sg[:], in_=v[:], func=AF.Sigmoid,
                                     bias=small["b2"][:, :1])
                nc.vector.tensor_scalar_mul(out=w3s[:, k * CH:(k + 1) * CH],
                                            in0=sg[:], scalar1=small["w3"][:, :1])
            alps = psum.tile([128, T1], F32, tag="alps")
            for i in range(T1):
                w = min(128, F1 - i * 128)
                if w <= 0:
                    break
                nc.tensor.matmul(out=alps[:w, i:i + 1],
                                 lhsT=w3s[:, i * 128:i * 128 + w], rhs=ones_c[:],
                                 start=True, stop=True)
            nc.scalar.copy(out=albuf[:], in_=alps[:])
            gps = psum.tile([BC, 128], F32, tag="gps")
            for i in range(T1):
                mt = work.tile([128, BC], BF16, tag="mt")
                nc.vector.tensor_scalar_mul(
                    out=mt[:], in0=bm_t[:, i * BC:(i + 1) * BC],
                    scalar1=albuf[:, i:i + 1])
                tpp = psum.tile([128, 128], BF16, tag="tpp")
                nc.tensor.transpose(out=tpp[:],
                                    in_=outt[:, i * 128:(i + 1) * 128],
                                    identity=identb[:])
                onat = work.tile([128, 128], BF16, tag="onat")
                nc.scalar.copy(out=onat[:], in_=tpp[:])
                nc.tensor.matmul(out=gps[:], lhsT=mt[:], rhs=onat[:],
                                 start=(i == 0), stop=(i == T1 - 1))
            gsb = work.tile([BC, 128], BF16, tag="gsb")
            nc.scalar.copy(out=gsb[:], in_=gps[:])
            gtp = psum.tile([128, BC], BF16, tag="gtp")
            nc.tensor.transpose(out=gtp[:], in_=gsb[:],
                                identity=identb[:BC, :BC])
            g_t = work.tile([128, BC], BF16, tag="g_t")
            nc.scalar.copy(out=g_t[:], in_=gtp[:])
            ghps = psum.tile([128, BC], F32, tag="ghps")
            nc.tensor.matmul(out=ghps[:], lhsT=wtr0_t[:], rhs=loc[:], start=True,
                             stop=False)
            nc.tensor.matmul(out=ghps[:], lhsT=wtr1_t[:], rhs=g_t[:], start=False,
                             stop=True)
            ghsb = work.tile([128, BC], F32, tag="ghsb")
            nc.scalar.add(out=ghsb[:], in_=ghps[:], add=small["btr"][:, :1])
            nc.sync.dma_start(out=ght_o[:], in_=ghsb[:])
            ppsum.__exit__(None, None, None)
    nc.compile()
    return nc


def build_launch_b():
    nc = bacc.Bacc(None)
    dp = nc.declare_dram_parameter
    ghtT = dp("ghtT", [128, B], BF16, isOutput=False)
    itemT = dp("itemT", [128, VSP], BF16, isOutput=False)
    out = dp("logits", [B, VSP], BF16, isOutput=True)
    with tile.TileContext(nc) as tc:
        with (
            tc.tile_pool(name="const", bufs=1) as const,
            tc.tile_pool(name="stream", bufs=4) as stream,
            tc.tile_pool(name="work", bufs=4) as work,
            tc.tile_pool(name="psum", bufs=4, space="PSUM") as psum,
        ):
            gh = const.tile([128, B], BF16)
            nc.sync.dma_start(out=gh[:], in_=ghtT[:])
            ob0 = const.tile([128, VSP], BF16, tag="ob0")
            ob1 = const.tile([128, VSP], BF16, tag="ob1")
            ob = [ob0, ob1]
            CW = 2560  # 5 psum-chunks per stream tile
            for g in range(VSP // CW):
                it = stream.tile([128, CW], BF16, tag="it")
                nc.sync.dma_start(out=it[:], in_=itemT[:, g * CW:(g + 1) * CW])
                for k in range(CW // 512):
                    c = g * (CW // 512) + k
                    for bh in range(2):
                        ps = psum.tile([128, 512], F32, tag="ps")
                        nc.tensor.matmul(out=ps[:],
                                         lhsT=gh[:, bh * 128:(bh + 1) * 128],
                                         rhs=it[:, k * 512:(k + 1) * 512],
                                         start=True, stop=True)
                        dst = ob[bh][:, c * 512:(c + 1) * 512]
                        if (c + bh) % 2 == 0:
                            nc.scalar.activation(out=dst, in_=ps[:],
                                                 func=AF.Relu)
                        else:
                            nc.vector.tensor_scalar_max(out=dst, in0=ps[:],
                                                        scalar1=0.0)
            for bh in range(2):
                nc.sync.dma_start(out=out[bh * 128:(bh + 1) * 128, :],
                                  in_=ob[bh][:])
    nc.compile()
    return nc


def _prep_core(c, h_iids, adj_entity, adj_relation, item_emb, rel_emb):
    h = h_iids[c * BC:(c + 1) * BC].astype(np.int64)  # [32, 50]
    hsm = np.ascontiguousarray(h.T).reshape(-1)  # s-major: tok = s*BC + b
    e1 = adj_entity[hsm].reshape(-1)  # [12800] flat, s-major tokens
    e2 = adj_entity[e1].reshape(-1, NB)  # [12800, 8]
    r1 = adj_relation[e1].reshape(-1, NB)  # [12800, 8]
    r0 = adj_relation[hsm].reshape(-1, NB)  # [1600, 8]

    npad0 = T0 * 128  # 13312
    e1p = np.zeros(npad0, np.int64)
    e1p[:BC * S * NB] = e1
    e2p = np.zeros((npad0, NB), np.int64)
    e2p[:e2.shape[0]] = e2
    r1p = np.zeros((npad0, NB), np.int64)
    r1p[:r1.shape[0]] = r1

    u = np.arange(T0)
    p = np.arange(128)
    perm = 1024 * (u[:, None] // 8) + 8 * p[None, :] + (u[:, None] % 8)

    self0 = item_emb[e1p[perm]]  # [104,128,128]
    nb0 = item_emb[e2p[perm]].reshape(T0, 128, NB * D)
    rel0 = rel_emb[r1p[perm]].reshape(T0, 128, NB * D)
    stream0 = np.concatenate([self0, rel0, nb0], axis=2)  # [104,128,2176]
    stream0 = np.ascontiguousarray(
        stream0.transpose(1, 0, 2)).astype(ml_dtypes.bfloat16)

    hf = np.zeros(F1P, np.int64)
    hf[:F1] = hsm
    r0p = np.zeros((F1P, NB), np.int64)
    r0p[:F1] = r0
    self1 = item_emb[hf].reshape(T1, 128, D)
    rel1 = rel_emb[r0p].reshape(T1, 128, NB * D)
    stream1 = np.concatenate([self1, rel1], axis=2)  # [13,128,1152]
    stream1 = np.ascontiguousarray(
        stream1.transpose(1, 0, 2)).astype(ml_dtypes.bfloat16)

    fl = np.arange(F1P)
    bmask = np.zeros((F1P, BC), np.float32)
    valid = fl < F1
    bmask[valid, fl[valid] % BC] = 1.0
    bmask = np.ascontiguousarray(
        bmask.reshape(T1, 128, BC).transpose(1, 0, 2).reshape(128, T1 * BC)
    ).astype(ml_dtypes.bfloat16)
    return dict(stream0=stream0, stream1=stream1, bmask=bmask)


def kernel(h_iids, a_iids, adj_entity, adj_relation, item_emb, rel_emb,
           Wa, ba, Wt, bt, Wih, Whh, bih, bhh,
           W1, b1, W2, b2, W3, Wtr, btr):
    h_iids = np.asarray(h_iids)
    adj_entity = np.asarray(adj_entity)
    adj_relation = np.asarray(adj_relation)
    item_emb = np.asarray(item_emb, np.float32)
    rel_emb = np.asarray(rel_emb, np.float32)

    if "a" not in _CACHE:
        _CACHE["a"] = build_launch_a()
    if "b" not in _CACHE:
        _CACHE["b"] = build_launch_b()
    nc_a, nc_b = _CACHE["a"], _CACHE["b"]

    bf = lambda x: np.ascontiguousarray(np.asarray(x, np.float32)).astype(
        ml_dtypes.bfloat16)
    col = lambda x: np.ascontiguousarray(np.asarray(x, np.float32).reshape(-1, 1))
    bihf = np.asarray(bih, np.float32)
    bhhf = np.asarray(bhh, np.float32)
    # ba shifts all pre-softmax scores equally within each softmax group, so it
    # cancels; it is intentionally unused.
    weights = dict(
        wa_b=bf(np.broadcast_to(np.asarray(Wa, np.float32).reshape(1, D),
                                (128, D))),
        wt=bf(Wt), bt=col(bt),
        wih=bf(Wih), whh=bf(Whh),
        biasr=col(bihf[:D] + bhhf[:D]),
        biasz=col(bihf[D:2 * D] + bhhf[D:2 * D]),
        nbiasz=col(-(bihf[D:2 * D] + bhhf[D:2 * D])),
        biasin=col(bihf[2 * D:]),
        biashn=col(bhhf[2 * D:]),
        w1=bf(W1), b1=col(b1), w2=bf(W2), b2=col(b2), w3=col(W3),
        wtr0=bf(np.asarray(Wtr, np.float32)[:D]),
        wtr1=bf(np.asarray(Wtr, np.float32)[D:]),
        btr=col(btr),
    )
    in_maps = []
    for c in range(NCORE):
        m = _prep_core(c, h_iids, adj_entity, adj_relation, item_emb, rel_emb)
        m.update(weights)
        in_maps.append(m)
    res_a = _run(nc_a, in_maps, "A")
    ghtT = np.concatenate([res_a.results[c]["ghtT"] for c in range(NCORE)],
                          axis=1).astype(np.float32)  # [128, 256]

    itemT_full = np.ascontiguousarray(item_emb.T)  # [128, 100000]
    ght_b = np.ascontiguousarray(ghtT).astype(ml_dtypes.bfloat16)
    in_maps_b = []
    for c in range(NCORE):
        sl = np.zeros((128, VSP), np.float32)
        sl[:, :VS] = itemT_full[:, c * VS:(c + 1) * VS]
        in_maps_b.append({"ghtT": ght_b, "itemT": sl.astype(ml_dtypes.bfloat16)})
    res_b = _run(nc_b, in_maps_b, "B")
    logits = np.concatenate(
        [np.asarray(res_b.results[c]["logits"][:, :VS], np.float32)
         for c in range(NCORE)], axis=1)
    return logits


# revision 12
# speedup vs baseline: 1.1984x; 1.1984x over previous
"""Trainium2 Bass kernel for nn_KASR_66005057405539 (KGAT-style recommender).

Strategy (8 NeuronCores):
- Host resolves the 2-hop KG index chains and materializes per-token bf16
  embedding streams in fused tile layout (pure indexing/layout work).
- Launch A (batch-sharded, 32 batches/core): 2 attention-aggregation hops
  processed as 4-tile supergroups (bf16 DVE 2x ops, fold-tree reductions,
  one wide Wt matmul per group), 50-step GRU, attention pooling
  -> ghtT [128d, 32b] per core.
- Launch B (vocab-sharded, 12500 items/core): logits = relu(ght @ item_emb.T)
  in bf16.
All floating-point math runs on device.
"""

import sys

sys.path.insert(0, "/root/problem")
import ml_dtypes
import numpy as np

import concourse.bacc as bacc
import concourse.mybir as mybir
import concourse.tile as tile
from concourse.bass_utils import run_bass_kernel_spmd
from concourse.masks import make_identity

F32 = mybir.dt.float32
BF16 = mybir.dt.bfloat16
AF = mybir.ActivationFunctionType
ALU = mybir.AluOpType

B, S, NB, D = 256, 50, 8, 128
N_ITEMS, N_RELS = 100000, 200
NCORE = 8
BC = B // NCORE  # 32 batches per core
F1 = BC * S  # 1600 hop-1 tokens per core
T1 = 13  # hop-1 tiles (1600 -> pad 1664)
F1P = T1 * 128  # 1664
T0 = T1 * 8  # 104 hop-0 tiles (13312 tokens)
VS = N_ITEMS // NCORE  # 12500 vocab per core
VSP = 25 * 512  # 12800 padded
FW = 2176  # fused stream row: self(128) | rel(1024) | nb(1024)
FW1 = 1152  # hop-1 stream row: self(128) | rel(1024)

_CACHE = {}
PROFILE = {}


def _run(nc, in_maps, label):
    import os

    trace = os.environ.get("KASR_PROFILE") == "1"
    if trace:
        try:
            r = run_bass_kernel_spmd(nc, in_maps, list(range(NCORE)), trace=True)
            PROFILE[label] = r.exec_time_ns
            return r
        except Exception:
            PROFILE[label] = None
    return run_bass_kernel_spmd(nc, in_maps, list(range(NCORE)))


def _hop_group(nc, pools, G, st3, nbv, wa_b, wt_t, bt_c, identb, out_fn):
    """One supergroup of G 128-token tiles (token-major, bf16).

    st3: stream view [128, G, FW-or-FW1] (self | rel [| nb]).
    nbv: neighbor view [128, G, 1024] (from stream or from neib0).
    out_fn(ps): consume the D-major Wt output PSUM [128, G*128].
    """
    stream, work, psum = pools
    selfv = st3[:, :, 0:D]
    relv = st3[:, :, D:D + NB * D]

    sfw = work.tile([128, G * D], BF16, tag="sfw")
    sfw3 = sfw[:].rearrange("p (u d) -> p u d", u=G)
    wab_b = wa_b[:].unsqueeze(1).broadcast_to([128, G, D])
    nc.vector.tensor_tensor(out=sfw3, in0=selfv, in1=wab_b, op=ALU.mult)

    prod = work.tile([128, G * NB * D], BF16, tag="prod")
    prod3 = prod[:].rearrange("p (u f) -> p u f", u=G)
    nc.vector.tensor_tensor(out=prod3, in0=relv, in1=nbv, op=ALU.mult)

    # p2 = prod * self*Wa (broadcast over n)
    p2 = work.tile([128, G * NB * D], BF16, tag="p2")
    p2v = p2[:].rearrange("p (u n d) -> p u n d", u=G, n=NB)
    prodv = prod[:].rearrange("p (u n d) -> p u n d", u=G, n=NB)
    sfw_b = sfw3.unsqueeze(2).broadcast_to([128, G, NB, D])
    nc.vector.tensor_tensor(out=p2v, in0=prodv, in1=sfw_b, op=ALU.mult)

    # att = sum_d p2 : fold d 128->64->32, then reduce
    f1 = work.tile([128, G * NB * 64], BF16, tag="f1")
    f1v = f1[:].rearrange("p (g s) -> p g s", s=64)
    nc.vector.tensor_tensor(out=f1v, in0=p2v[:, :, :, 0:64].rearrange(
        "p u n s -> p (u n) s"), in1=p2v[:, :, :, 64:128].rearrange(
        "p u n s -> p (u n) s"), op=ALU.add)
    f2 = work.tile([128, G * NB * 32], BF16, tag="f2")
    f2v = f2[:].rearrange("p (g s) -> p g s", s=32)
    nc.vector.tensor_tensor(out=f2v, in0=f1v[:, :, 0:32], in1=f1v[:, :, 32:64],
                            op=ALU.add)
    att = work.tile([128, G * NB], F32, tag="att")
    nc.vector.tensor_reduce(out=att[:], in_=f2v, axis=mybir.AxisListType.X,
                            op=ALU.add)

    # softmax over n (groups of 8): exp + per-tile row-sum accumulators
    e = work.tile([128, G * NB], BF16, tag="e")
    se = work.tile([128, G], F32, tag="se")
    for u in range(G):
        nc.scalar.activation(out=e[:, u * NB:(u + 1) * NB],
                             in_=att[:, u * NB:(u + 1) * NB], func=AF.Exp,
                             accum_out=se[:, u:u + 1])
    rec = work.tile([128, G], F32, tag="rec")
    nc.vector.reciprocal(out=rec[:], in_=se[:])
    alpha = work.tile([128, G * NB], BF16, tag="alpha")
    av = alpha[:].rearrange("p (u n) -> p u n", u=G)
    ev = e[:].rearrange("p (u n) -> p u n", u=G)
    rec_b = rec[:].unsqueeze(2).broadcast_to([128, G, NB])
    nc.vector.tensor_tensor(out=av, in0=ev, in1=rec_b, op=ALU.mult)
    # pair-expand alpha along d so the weighting multiply keeps a packed
    # 2-byte last dim (DVE 2x)
    alphaX = work.tile([128, G * NB * 2], BF16, tag="alphaX")
    aXv = alphaX[:].rearrange("p (g two) -> p g two", two=2)
    a_b = alpha[:].unsqueeze(2).broadcast_to([128, G * NB, 2])
    nc.vector.tensor_scalar_mul(out=aXv, in0=a_b, scalar1=1.0)

    wnb = work.tile([128, G * NB * D], BF16, tag="wnb")
    wnbv = wnb[:].rearrange("p (u g s two) -> p u g s two", u=G, g=NB, two=2)
    nb4 = nbv.rearrange("p u (g s two) -> p u g s two", g=NB, two=2)
    aX_b = alphaX[:].rearrange("p (u g two) -> p u g two", u=G, two=2).unsqueeze(
        3).broadcast_to([128, G, NB, 64, 2])
    nc.vector.tensor_tensor(out=wnbv, in0=nb4, in1=aX_b, op=ALU.mult)

    # agg = sum_n wnb : fold n 8->4->2->1
    w4 = wnb[:].rearrange("p (u n d) -> p u n d", u=G, n=NB)
    t1 = work.tile([128, G * 4 * D], BF16, tag="t1")
    t1v = t1[:].rearrange("p (u n d) -> p u n d", u=G, n=4)
    nc.vector.tensor_tensor(out=t1v, in0=w4[:, :, 0:4, :], in1=w4[:, :, 4:8, :],
                            op=ALU.add)
    t2 = work.tile([128, G * 2 * D], BF16, tag="t2")
    t2v = t2[:].rearrange("p (u n d) -> p u n d", u=G, n=2)
    nc.vector.tensor_tensor(out=t2v, in0=t1v[:, :, 0:2, :], in1=t1v[:, :, 2:4, :],
                            op=ALU.add)
    ags = work.tile([128, G * D], BF16, tag="ags")
    agsv = ags[:].rearrange("p (u d) -> p u d", u=G)
    # fold last pair and add self in two ops: agg2 = n0 + n1 ; ags = agg2 + self
    agg = work.tile([128, G * D], BF16, tag="agg")
    aggv = agg[:].rearrange("p (u d) -> p u d", u=G)
    nc.vector.tensor_tensor(out=aggv, in0=t2v[:, :, 0, :], in1=t2v[:, :, 1, :],
                            op=ALU.add)
    nc.vector.tensor_tensor(out=agsv, in0=aggv, in1=selfv, op=ALU.add)

    # (self+agg) @ Wt : transpose each tile to D-major, one wide matmul
    tps = work.tile([128, G * D], BF16, tag="tps")
    for u in range(G):
        tp = psum.tile([128, D], BF16, tag="tp")
        nc.tensor.transpose(out=tp[:], in_=ags[:, u * D:(u + 1) * D],
                            identity=identb[:])
        nc.scalar.copy(out=tps[:, u * D:(u + 1) * D], in_=tp[:])
    mm = psum.tile([128, G * D], F32, tag="mm")
    nc.tensor.matmul(out=mm[:], lhsT=wt_t[:], rhs=tps[:], start=True, stop=True)
    out_fn(mm)


def build_launch_a():
    nc = bacc.Bacc(None)
    dp = nc.declare_dram_parameter
    stream0 = dp("stream0", [128, T0, FW], BF16, isOutput=False)
    stream1 = dp("stream1", [128, T1, FW1], BF16, isOutput=False)
    wa_p = dp("wa_b", [128, D], BF16, isOutput=False)
    wt_p = dp("wt", [D, D], BF16, isOutput=False)
    bt_p = dp("bt", [128, 1], F32, isOutput=False)
    wih_p = dp("wih", [D, 3 * D], BF16, isOutput=False)
    whh_p = dp("whh", [D, 3 * D], BF16, isOutput=False)
    biasr_p = dp("biasr", [128, 1], F32, isOutput=False)
    biasz_p = dp("biasz", [128, 1], F32, isOutput=False)
    nbiasz_p = dp("nbiasz", [128, 1], F32, isOutput=False)
    biasin_p = dp("biasin", [128, 1], F32, isOutput=False)
    biashn_p = dp("biashn", [128, 1], F32, isOutput=False)
    w1_p = dp("w1", [D, D], BF16, isOutput=False)
    b1_p = dp("b1", [128, 1], F32, isOutput=False)
    w2_p = dp("w2", [D, D], BF16, isOutput=False)
    b2_p = dp("b2", [128, 1], F32, isOutput=False)
    w3_p = dp("w3", [128, 1], F32, isOutput=False)
    wtr0_p = dp("wtr0", [D, D], BF16, isOutput=False)
    wtr1_p = dp("wtr1", [D, D], BF16, isOutput=False)
    btr_p = dp("btr", [128, 1], F32, isOutput=False)
    bmask_p = dp("bmask", [128, T1 * BC], BF16, isOutput=False)
    ght_o = dp("ghtT", [128, BC], F32, isOutput=True)

    with tile.TileContext(nc) as tc:
        with (
            tc.tile_pool(name="const", bufs=1) as const,
            tc.tile_pool(name="stream", bufs=3) as stream,
            tc.tile_pool(name="work", bufs=2) as work,
        ):
            ident = const.tile([128, 128], F32)
            make_identity(nc, ident[:])
            identb = const.tile([128, 128], BF16)
            nc.gpsimd.memset(identb[:], 0.0)
            nc.vector.tensor_scalar_add(out=identb[:], in0=ident[:], scalar1=0.0)

            def load(name, p, shape, dt):
                t = const.tile(shape, dt, tag=name)
                nc.sync.dma_start(out=t[:], in_=p[:])
                return t

            wa_b = load("wa_b", wa_p, [128, D], BF16)
            wt_t = load("wt", wt_p, [D, D], BF16)
            bt_c = load("bt", bt_p, [128, 1], F32)
            wih_t = load("wih", wih_p, [D, 3 * D], BF16)
            whh_t = load("whh", whh_p, [D, 3 * D], BF16)
            small = {}
            for nm, p in (("biasr", biasr_p), ("biasz", biasz_p),
                          ("nbiasz", nbiasz_p), ("biasin", biasin_p),
                          ("biashn", biashn_p), ("b1", b1_p), ("b2", b2_p),
                          ("w3", w3_p), ("btr", btr_p)):
                small[nm] = load(nm, p, [128, 1], F32)
            w1_t = load("w1", w1_p, [D, D], BF16)
            w2_t = load("w2", w2_p, [D, D], BF16)
            wtr0_t = load("wtr0", wtr0_p, [D, D], BF16)
            wtr1_t = load("wtr1", wtr1_p, [D, D], BF16)
            bm_t = load("bmask", bmask_p, [128, T1 * BC], BF16)
            ones_c = const.tile([128, 1], BF16)
            nc.gpsimd.memset(ones_c[:], 1.0)
            zero32 = const.tile([128, BC], BF16)
            nc.gpsimd.memset(zero32[:], 0.0)

            neib0 = const.tile([128, T0 * D], BF16)  # hop-0 out, token-major
            xt = const.tile([128, F1P], BF16)  # hop-1 out, D-major
            outt = const.tile([128, F1P], BF16)  # GRU out, D-major
            nc.gpsimd.memset(outt[:, F1:], 0.0)
            w3s = const.tile([128, F1P], BF16)
            albuf = const.tile([128, T1], F32)

            hpsum = tc.tile_pool(name="hpsum", bufs=1, space="PSUM")
            gpsum = tc.tile_pool(name="gpsum", bufs=1, space="PSUM")
            psum = hpsum.__enter__()
            gps_pool = gpsum.__enter__()
            pools = (stream, work, psum)
            G = 4

            def hop0_group(j):
                st = stream.tile([128, G * FW], BF16, tag="st0")
                st3 = st[:].rearrange("p (u f) -> p u f", u=G)
                nc.sync.dma_start(out=st3, in_=stream0[:, j * G:(j + 1) * G])
                nbv = st3[:, :, D + NB * D:FW]

                def out0(mm, j=j):
                    nbt = work.tile([128, G * D], BF16, tag="nbt")
                    nc.scalar.add(out=nbt[:], in_=mm[:], add=bt_c[:, :1])
                    for u in range(G):
                        tb = psum.tile([128, D], BF16, tag="tb")
                        nc.tensor.transpose(out=tb[:],
                                            in_=nbt[:, u * D:(u + 1) * D],
                                            identity=identb[:])
                        g = (j * G + u) * D
                        nc.scalar.copy(out=neib0[:, g:g + D], in_=tb[:])

                _hop_group(nc, pools, G, st3, nbv, wa_b, wt_t, bt_c,
                           identb, out0)

            def hop1_group(lo, G1):
                st = stream.tile([128, 4 * FW1], BF16, tag="st1")
                st3 = st[:].rearrange("p (u f) -> p u f", u=4)[:, :G1]
                nc.sync.dma_start(out=st3, in_=stream1[:, lo:lo + G1])
                nbv = neib0[:, lo * NB * D:(lo + G1) * NB * D].rearrange(
                    "p (u f) -> p u f", u=G1)

                def out1(mm, lo=lo, G1=G1):
                    nc.scalar.add(out=xt[:, lo * D:(lo + G1) * D],
                                  in_=mm[:], add=bt_c[:, :1])

                _hop_group(nc, pools, G1, st3, nbv, wa_b, wt_t, bt_c,
                           identb, out1)

            def gru_step(t):
                x_t = xt[:, t * BC:(t + 1) * BC]
                h_prev = (outt[:, (t - 1) * BC:t * BC] if t > 0 else zero32[:])
                psr = gps_pool.tile([128, BC], F32, tag="psr")
                nc.tensor.matmul(out=psr[:], lhsT=wih_t[:, 0:D], rhs=x_t,
                                 start=True, stop=False)
                nc.tensor.matmul(out=psr[:], lhsT=whh_t[:, 0:D], rhs=h_prev,
                                 start=False, stop=True)
                r = work.tile([128, BC], BF16, tag="gr")
                nc.scalar.activation(out=r[:], in_=psr[:], func=AF.Sigmoid,
                                     bias=small["biasr"][:, :1])
                psz = gps_pool.tile([128, BC], F32, tag="psz")
                nc.tensor.matmul(out=psz[:], lhsT=wih_t[:, D:2 * D], rhs=x_t,
                                 start=True, stop=False)
                nc.tensor.matmul(out=psz[:], lhsT=whh_t[:, D:2 * D], rhs=h_prev,
                                 start=False, stop=True)
                z = work.tile([128, BC], BF16, tag="gz")
                nc.scalar.activation(out=z[:], in_=psz[:], func=AF.Sigmoid,
                                     bias=small["biasz"][:, :1])
                zm = work.tile([128, BC], BF16, tag="gzm")
                nc.scalar.activation(out=zm[:], in_=psz[:], func=AF.Sigmoid,
                                     scale=-1.0, bias=small["nbiasz"][:, :1])
                zh = work.tile([128, BC], BF16, tag="zh")
                nc.vector.tensor_tensor(out=zh[:], in0=z[:], in1=h_prev,
                                        op=ALU.mult)
                psin = gps_pool.tile([128, BC], F32, tag="psin")
                nc.tensor.matmul(out=psin[:], lhsT=wih_t[:, 2 * D:], rhs=x_t,
                                 start=True, stop=True)
                pshn = gps_pool.tile([128, BC], F32, tag="pshn")
                nc.tensor.matmul(out=pshn[:], lhsT=whh_t[:, 2 * D:], rhs=h_prev,
                                 start=True, stop=True)
                psinb = work.tile([128, BC], BF16, tag="psinb")
                nc.vector.tensor_scalar_add(out=psinb[:], in0=psin[:],
                                            scalar1=small["biasin"][:, :1])
                rhn = work.tile([128, BC], BF16, tag="rhn")
                nc.vector.scalar_tensor_tensor(
                    out=rhn[:], in0=pshn[:], scalar=small["biashn"][:, :1],
                    in1=r[:], op0=ALU.add, op1=ALU.mult)
                pre = work.tile([128, BC], BF16, tag="pre")
                nc.vector.tensor_tensor(out=pre[:], in0=rhn[:], in1=psinb[:],
                                        op=ALU.add)
                n_sb = work.tile([128, BC], BF16, tag="n")
                nc.scalar.activation(out=n_sb[:], in_=pre[:], func=AF.Tanh)
                nzm = work.tile([128, BC], BF16, tag="nzm")
                nc.vector.tensor_tensor(out=nzm[:], in0=n_sb[:], in1=zm[:],
                                        op=ALU.mult)
                nc.vector.tensor_tensor(out=outt[:, t * BC:(t + 1) * BC],
                                        in0=nzm[:], in1=zh[:], op=ALU.add)

            # interleave: hop-0 groups feed hop-1 groups feed GRU steps so the
            # GRU dependency chain hides under hop DVE work
            done0 = 0
            for lo, G1 in ((0, 4), (4, 4), (8, 4), (12, 1)):
                need0 = 2 * (lo + G1)
                while done0 < need0:
                    hop0_group(done0)
                    done0 += 1
                hop1_group(lo, G1)
                for t in range(lo * 4, min(S, (lo + G1) * 4)):
                    gru_step(t)
            gpsum.__exit__(None, None, None)
            hpsum.__exit__(None, None, None)

            # ---- attention pooling ----
            ppsum = tc.tile_pool(name="ppsum", bufs=1, space="PSUM")
            psum = ppsum.__enter__()
            loc = work.tile([128, BC], BF16, tag="loc")
            nc.scalar.copy(out=loc[:], in_=outt[:, (S - 1) * BC:S * BC])
            q1ps = psum.tile([128, BC], F32, tag="q1")
            nc.tensor.matmul(out=q1ps[:], lhsT=w1_t[:], rhs=loc[:], start=True,
                             stop=True)
            q1s = work.tile([128, BC], BF16, tag="q1s")
            nc.scalar.add(out=q1s[:], in_=q1ps[:], add=small["b1"][:, :1])
            CH = 10 * BC  # 320 cols = 10 s-steps per chunk (s-major layout)
            for k in range(5):
                ps = psum.tile([128, CH], F32, tag="q2")
                nc.tensor.matmul(out=ps[:], lhsT=w2_t[:],
                                 rhs=outt[:, k * CH:(k + 1) * CH],
                                 start=True, stop=True)
                v = work.tile([128, CH], BF16, tag="v")
                q1b = q1s[:].unsqueeze(1).broadcast_to([128, 10, BC])
                vv = v[:].rearrange("p (s b) -> p s b", b=BC)
                psv = ps[:].rearrange("p (s b) -> p s b", b=BC)
                nc.vector.tensor_tensor(out=vv, in0=psv, in1=q1b, op=ALU.add)
                sg = work.tile([128, CH], BF16, tag="sg")
                nc.scalar.activation(out=sg[:], in_=v[:], func=AF.Sigmoid,
                                     bias=small["b2"][:, :1])
                nc.vector.tensor_scalar_mul(out=w3s[:, k * CH:(k + 1) * CH],
                                            in0=sg[:], scalar1=small["w3"][:, :1])
            alps = psum.tile([128, T1], F32, tag="alps")
            for i in range(T1):
                w = min(128, F1 - i * 128)
                if w <= 0:
                    break
                nc.tensor.matmul(out=alps[:w, i:i + 1],
                                 lhsT=w3s[:, i * 128:i * 128 + w], rhs=ones_c[:],
                                 start=True, stop=True)
            nc.scalar.copy(out=albuf[:], in_=alps[:])
            gps = psum.tile([BC, 128], F32, tag="gps")
            for i in range(T1):
                mt = work.tile([128, BC], BF16, tag="mt")
                nc.vector.tensor_scalar_mul(
                    out=mt[:], in0=bm_t[:, i * BC:(i + 1) * BC],
                    scalar1=albuf[:, i:i + 1])
                tpp = psum.tile([128, 128], BF16, tag="tpp")
                nc.tensor.transpose(out=tpp[:],
                                    in_=outt[:, i * 128:(i + 1) * 128],
                                    identity=identb[:])
                onat = work.tile([128, 128], BF16, tag="onat")
                nc.scalar.copy(out=onat[:], in_=tpp[:])
                nc.tensor.matmul(out=gps[:], lhsT=mt[:], rhs=onat[:],
                                 start=(i == 0), stop=(i == T1 - 1))
            gsb = work.tile([BC, 128], BF16, tag="gsb")
            nc.scalar.copy(out=gsb[:], in_=gps[:])
            gtp = psum.tile([128, BC], BF16, tag="gtp")
            nc.tensor.transpose(out=gtp[:], in_=gsb[:],
                                identity=identb[:BC, :BC])
            g_t = work.tile([128, BC], BF16, tag="g_t")
            nc.scalar.copy(out=g_t[:], in_=gtp[:])
            ghps = psum.tile([128, BC], F32, tag="ghps")
            nc.tensor.matmul(out=ghps[:], lhsT=wtr0_t[:], rhs=loc[:], start=True,
                             stop=False)
            nc.tensor.matmul(out=ghps[:], lhsT=wtr1_t[:], rhs=g_t[:], start=False,
                             stop=True)
            ghsb = work.tile([128, BC], F32, tag="ghsb")
            nc.scalar.add(out=ghsb[:], in_=ghps[:], add=small["btr"][:, :1])
            nc.sync.dma_start(out=ght_o[:], in_=ghsb[:])
            ppsum.__exit__(None, None, None)
    nc.compile()
    return nc


def build_launch_b():
    nc = bacc.Bacc(None)
    dp = nc.declare_dram_parameter
    ghtT = dp("ghtT", [128, B], BF16, isOutput=False)
    itemT = dp("itemT", [128, VSP], BF16, isOutput=False)
    out = dp("logits", [B, VSP], BF16, isOutput=True)
    with tile.TileContext(nc) as tc:
        with (
            tc.tile_pool(name="const", bufs=1) as const,
            tc.tile_pool(name="stream", bufs=4) as stream,
            tc.tile_pool(name="work", bufs=4) as work,
            tc.tile_pool(name="psum", bufs=4, space="PSUM") as psum,
        ):
            gh = const.tile([128, B], BF16)
            nc.sync.dma_start(out=gh[:], in_=ghtT[:])
            ob0 = const.tile([128, VSP], BF16, tag="ob0")
            ob1 = const.tile([128, VSP], BF16, tag="ob1")
            ob = [ob0, ob1]
            CW = 2560  # 5 psum-chunks per stream tile
            for g in range(VSP // CW):
                it = stream.tile([128, CW], BF16, tag="it")
                nc.sync.dma_start(out=it[:], in_=itemT[:, g * CW:(g + 1) * CW])
                for k in range(CW // 512):
                    c = g * (CW // 512) + k
                    for bh in range(2):
                        ps = psum.tile([128, 512], F32, tag="ps")
                        nc.tensor.matmul(out=ps[:],
                                         lhsT=gh[:, bh * 128:(bh + 1) * 128],
                                         rhs=it[:, k * 512:(k + 1) * 512],
                                         start=True, stop=True)
                        dst = ob[bh][:, c * 512:(c + 1) * 512]
                        if (c + bh) % 2 == 0:
                            nc.scalar.activation(out=dst, in_=ps[:],
                                                 func=AF.Relu)
                        else:
                            nc.vector.tensor_scalar_max(out=dst, in0=ps[:],
                                                        scalar1=0.0)
            for bh in range(2):
                nc.sync.dma_start(out=out[bh * 128:(bh + 1) * 128, :],
                                  in_=ob[bh][:])
    nc.compile()
    return nc


def _prep_core(c, h_iids, adj_entity, adj_relation, item_emb, rel_emb):
    h = h_iids[c * BC:(c + 1) * BC].astype(np.int64)  # [32, 50]
    hsm = np.ascontiguousarray(h.T).reshape(-1)  # s-major: tok = s*BC + b
    e1 = adj_entity[hsm].reshape(-1)  # [12800] flat, s-major tokens
    e2 = adj_entity[e1].reshape(-1, NB)  # [12800, 8]
    r1 = adj_relation[e1].reshape(-1, NB)  # [12800, 8]
    r0 = adj_relation[hsm].reshape(-1, NB)  # [1600, 8]

    npad0 = T0 * 128  # 13312
    e1p = np.zeros(npad0, np.int64)
    e1p[:BC * S * NB] = e1
    e2p = np.zeros((npad0, NB), np.int64)
    e2p[:e2.shape[0]] = e2
    r1p = np.zeros((npad0, NB), np.int64)
    r1p[:r1.shape[0]] = r1

    u = np.arange(T0)
    p = np.arange(128)
    perm = 1024 * (u[:, None] // 8) + 8 * p[None, :] + (u[:, None] % 8)

    self0 = item_emb[e1p[perm]]  # [104,128,128]
    nb0 = item_emb[e2p[perm]].reshape(T0, 128, NB * D)
    rel0 = rel_emb[r1p[perm]].reshape(T0, 128, NB * D)
    stream0 = np.concatenate([self0, rel0, nb0], axis=2)  # [104,128,2176]
    stream0 = np.ascontiguousarray(
        stream0.transpose(1, 0, 2)).astype(ml_dtypes.bfloat16)

    hf = np.zeros(F1P, np.int64)
    hf[:F1] = hsm
    r0p = np.zeros((F1P, NB), np.int64)
    r0p[:F1] = r0
    self1 = item_emb[hf].reshape(T1, 128, D)
    rel1 = rel_emb[r0p].reshape(T1, 128, NB * D)
    stream1 = np.concatenate([self1, rel1], axis=2)  # [13,128,1152]
    stream1 = np.ascontiguousarray(
        stream1.transpose(1, 0, 2)).astype(ml_dtypes.bfloat16)

    fl = np.arange(F1P)
    bmask = np.zeros((F1P, BC), np.float32)
    valid = fl < F1
    bmask[valid, fl[valid] % BC] = 1.0
    bmask = np.ascontiguousarray(
        bmask.reshape(T1, 128, BC).transpose(1, 0, 2).reshape(128, T1 * BC)
    ).astype(ml_dtypes.bfloat16)
    return dict(stream0=stream0, stream1=stream1, bmask=bmask)


def kernel(h_iids, a_iids, adj_entity, adj_relation, item_emb, rel_emb,
           Wa, ba, Wt, bt, Wih, Whh, bih, bhh,
           W1, b1, W2, b2, W3, Wtr, btr):
    h_iids = np.asarray(h_iids)
    adj_entity = np.asarray(adj_entity)
    adj_relation = np.asarray(adj_relation)
    item_emb = np.asarray(item_emb, np.float32)
    rel_emb = np.asarray(rel_emb, np.float32)

    if "a" not in _CACHE:
        _CACHE["a"] = build_launch_a()
    if "b" not in _CACHE:
        _CACHE["b"] = build_launch_b()
    nc_a, nc_b = _CACHE["a"], _CACHE["b"]

    bf = lambda x: np.ascontiguousarray(np.asarray(x, np.float32)).astype(
        ml_dtypes.bfloat16)
    col = lambda x: np.ascontiguousarray(np.asarray(x, np.float32).reshape(-1, 1))
    bihf = np.asarray(bih, np.float32)
    bhhf = np.asarray(bhh, np.float32)
    # ba shifts all pre-softmax scores equally within each softmax group, so it
    # cancels; it is intentionally unused.
    weights = dict(
        wa_b=bf(np.broadcast_to(np.asarray(Wa, np.float32).reshape(1, D),
                                (128, D))),
        wt=bf(Wt), bt=col(bt),
        wih=bf(Wih), whh=bf(Whh),
        biasr=col(bihf[:D] + bhhf[:D]),
        biasz=col(bihf[D:2 * D] + bhhf[D:2 * D]),
        nbiasz=col(-(bihf[D:2 * D] + bhhf[D:2 * D])),
        biasin=col(bihf[2 * D:]),
        biashn=col(bhhf[2 * D:]),
        w1=bf(W1), b1=col(b1), w2=bf(W2), b2=col(b2), w3=col(W3),
        wtr0=bf(np.asarray(Wtr, np.float32)[:D]),
        wtr1=bf(np.asarray(Wtr, np.float32)[D:]),
        btr=col(btr),
    )
    in_maps = []
    for c in range(NCORE):
        m = _prep_core(c, h_iids, adj_entity, adj_relation, item_emb, rel_emb)
        m.update(weights)
        in_maps.append(m)
    res_a = _run(nc_a, in_maps, "A")
    ghtT = np.concatenate([res_a.results[c]["ghtT"] for c in range(NCORE)],
                          axis=1).astype(np.float32)  # [128, 256]

    itemT_full = np.ascontiguousarray(item_emb.T)  # [128, 100000]
    ght_b = np.ascontiguousarray(ghtT).astype(ml_dtypes.bfloat16)
    in_maps_b = []
    for c in range(NCORE):
        sl = np.zeros((128, VSP), np.float32)
        sl[:, :VS] = itemT_full[:, c * VS:(c + 1) * VS]
        in_maps_b.append({"ghtT": ght_b, "itemT": sl.astype(ml_dtypes.bfloat16)})
    res_b = _run(nc_b, in_maps_b, "B")
    logits = np.concatenate(
        [np.asarray(res_b.results[c]["logits"][:, :VS], np.float32)
         for c in range(NCORE)], axis=1)
    return logits


# revision 13
# speedup vs baseline: 1.2214x; 1.0192x over previous
"""Trainium2 Bass kernel for nn_KASR_66005057405539 (KGAT-style recommender).

Strategy (8 NeuronCores):
- Host resolves the 2-hop KG index chains and materializes per-token bf16
  embedding streams in fused tile layout (pure indexing/layout work).
- Launch A (batch-sharded, 32 batches/core): 2 attention-aggregation hops
  processed as 4-tile supergroups (bf16 DVE 2x ops, fold-tree reductions,
  one wide Wt matmul per group), 50-step GRU, attention pooling
  -> ghtT [128d, 32b] per core.
- Launch B (vocab-sharded, 12500 items/core): logits = relu(ght @ item_emb.T)
  in bf16.
All floating-point math runs on device.
"""

import sys

sys.path.insert(0, "/root/problem")
import ml_dtypes
import numpy as np

import concourse.bacc as bacc
import concourse.mybir as mybir
import concourse.tile as tile
from concourse.bass_utils import run_bass_kernel_spmd
from concourse.masks import make_identity

F32 = mybir.dt.float32
BF16 = mybir.dt.bfloat16
AF = mybir.ActivationFunctionType
ALU = mybir.AluOpType

B, S, NB, D = 256, 50, 8, 128
N_ITEMS, N_RELS = 100000, 200
NCORE = 8
BC = B // NCORE  # 32 batches per core
F1 = BC * S  # 1600 hop-1 tokens per core
T1 = 13  # hop-1 tiles (1600 -> pad 1664)
F1P = T1 * 128  # 1664
T0 = T1 * 8  # 104 hop-0 tiles (13312 tokens)
VS = N_ITEMS // NCORE  # 12500 vocab per core
VSP = 25 * 512  # 12800 padded
FW = 2176  # fused stream row: self(128) | rel(1024) | nb(1024)
FW1 = 1152  # hop-1 stream row: self(128) | rel(1024)

_CACHE = {}
PROFILE = {}


def _run(nc, in_maps, label):
    import os

    trace = os.environ.get("KASR_PROFILE") == "1"
    if trace:
        try:
            r = run_bass_kernel_spmd(nc, in_maps, list(range(NCORE)), trace=True)
            PROFILE[label] = r.exec_time_ns
            return r
        except Exception:
            PROFILE[label] = None
    return run_bass_kernel_spmd(nc, in_maps, list(range(NCORE)))


def _hop_group(nc, pools, G, st3, nbv, wa_b, wt_t, bt_c, identb, out_fn):
    """One supergroup of G 128-token tiles (token-major, bf16).

    st3: stream view [128, G, FW-or-FW1] (self | rel [| nb]).
    nbv: neighbor view [128, G, 1024] (from stream or from neib0).
    out_fn(ps): consume the D-major Wt output PSUM [128, G*128].
    """
    stream, work, psum = pools
    selfv = st3[:, :, 0:D]
    relv = st3[:, :, D:D + NB * D]

    sfw = work.tile([128, G * D], BF16, tag="sfw")
    sfw3 = sfw[:].rearrange("p (u d) -> p u d", u=G)
    wab_b = wa_b[:].unsqueeze(1).broadcast_to([128, G, D])
    nc.vector.tensor_tensor(out=sfw3, in0=selfv, in1=wab_b, op=ALU.mult)

    prod = work.tile([128, G * NB * D], BF16, tag="prod")
    prod3 = prod[:].rearrange("p (u f) -> p u f", u=G)
    nc.vector.tensor_tensor(out=prod3, in0=relv, in1=nbv, op=ALU.mult)

    # p2 = prod * self*Wa (broadcast over n)
    p2 = work.tile([128, G * NB * D], BF16, tag="p2")
    p2v = p2[:].rearrange("p (u n d) -> p u n d", u=G, n=NB)
    prodv = prod[:].rearrange("p (u n d) -> p u n d", u=G, n=NB)
    sfw_b = sfw3.unsqueeze(2).broadcast_to([128, G, NB, D])
    nc.vector.tensor_tensor(out=p2v, in0=prodv, in1=sfw_b, op=ALU.mult)

    # att = sum_d p2 : fold d 128->64->32, then reduce
    f1 = work.tile([128, G * NB * 64], BF16, tag="f1")
    f1v = f1[:].rearrange("p (g s) -> p g s", s=64)
    nc.vector.tensor_tensor(out=f1v, in0=p2v[:, :, :, 0:64].rearrange(
        "p u n s -> p (u n) s"), in1=p2v[:, :, :, 64:128].rearrange(
        "p u n s -> p (u n) s"), op=ALU.add)
    f2 = work.tile([128, G * NB * 32], BF16, tag="f2")
    f2v = f2[:].rearrange("p (g s) -> p g s", s=32)
    nc.vector.tensor_tensor(out=f2v, in0=f1v[:, :, 0:32], in1=f1v[:, :, 32:64],
                            op=ALU.add)
    att = work.tile([128, G * NB], F32, tag="att")
    nc.vector.tensor_reduce(out=att[:], in_=f2v, axis=mybir.AxisListType.X,
                            op=ALU.add)

    # softmax over n (groups of 8): exp + per-tile row-sum accumulators
    e = work.tile([128, G * NB], BF16, tag="e")
    se = work.tile([128, G], F32, tag="se")
    for u in range(G):
        nc.scalar.activation(out=e[:, u * NB:(u + 1) * NB],
                             in_=att[:, u * NB:(u + 1) * NB], func=AF.Exp,
                             accum_out=se[:, u:u + 1])
    rec = work.tile([128, G], F32, tag="rec")
    nc.vector.reciprocal(out=rec[:], in_=se[:])
    alpha = work.tile([128, G * NB], BF16, tag="alpha")
    av = alpha[:].rearrange("p (u n) -> p u n", u=G)
    ev = e[:].rearrange("p (u n) -> p u n", u=G)
    rec_b = rec[:].unsqueeze(2).broadcast_to([128, G, NB])
    nc.vector.tensor_tensor(out=av, in0=ev, in1=rec_b, op=ALU.mult)
    # pair-expand alpha along d so the weighting multiply keeps a packed
    # 2-byte last dim (DVE 2x)
    alphaX = work.tile([128, G * NB * 2], BF16, tag="alphaX")
    aXv = alphaX[:].rearrange("p (g two) -> p g two", two=2)
    a_b = alpha[:].unsqueeze(2).broadcast_to([128, G * NB, 2])
    nc.vector.tensor_scalar_mul(out=aXv, in0=a_b, scalar1=1.0)

    wnb = work.tile([128, G * NB * D], BF16, tag="wnb")
    wnbv = wnb[:].rearrange("p (u g s two) -> p u g s two", u=G, g=NB, two=2)
    nb4 = nbv.rearrange("p u (g s two) -> p u g s two", g=NB, two=2)
    aX_b = alphaX[:].rearrange("p (u g two) -> p u g two", u=G, two=2).unsqueeze(
        3).broadcast_to([128, G, NB, 64, 2])
    nc.vector.tensor_tensor(out=wnbv, in0=nb4, in1=aX_b, op=ALU.mult)

    # agg = sum_n wnb : fold n 8->4->2->1
    w4 = wnb[:].rearrange("p (u n d) -> p u n d", u=G, n=NB)
    t1 = work.tile([128, G * 4 * D], BF16, tag="t1")
    t1v = t1[:].rearrange("p (u n d) -> p u n d", u=G, n=4)
    nc.vector.tensor_tensor(out=t1v, in0=w4[:, :, 0:4, :], in1=w4[:, :, 4:8, :],
                            op=ALU.add)
    t2 = work.tile([128, G * 2 * D], BF16, tag="t2")
    t2v = t2[:].rearrange("p (u n d) -> p u n d", u=G, n=2)
    nc.vector.tensor_tensor(out=t2v, in0=t1v[:, :, 0:2, :], in1=t1v[:, :, 2:4, :],
                            op=ALU.add)
    ags = work.tile([128, G * D], BF16, tag="ags")
    agsv = ags[:].rearrange("p (u d) -> p u d", u=G)
    # fold last pair and add self in two ops: agg2 = n0 + n1 ; ags = agg2 + self
    agg = work.tile([128, G * D], BF16, tag="agg")
    aggv = agg[:].rearrange("p (u d) -> p u d", u=G)
    nc.vector.tensor_tensor(out=aggv, in0=t2v[:, :, 0, :], in1=t2v[:, :, 1, :],
                            op=ALU.add)
    nc.vector.tensor_tensor(out=agsv, in0=aggv, in1=selfv, op=ALU.add)

    # (self+agg) @ Wt : transpose each tile to D-major, one wide matmul
    tps = work.tile([128, G * D], BF16, tag="tps")
    for u in range(G):
        tp = psum.tile([128, D], BF16, tag="tp")
        nc.tensor.transpose(out=tp[:], in_=ags[:, u * D:(u + 1) * D],
                            identity=identb[:])
        nc.scalar.copy(out=tps[:, u * D:(u + 1) * D], in_=tp[:])
    mm = psum.tile([128, G * D], F32, tag="mm")
    nc.tensor.matmul(out=mm[:], lhsT=wt_t[:], rhs=tps[:], start=True, stop=True)
    out_fn(mm)


def build_launch_a():
    nc = bacc.Bacc(None)
    dp = nc.declare_dram_parameter
    stream0 = dp("stream0", [128, T0, FW], BF16, isOutput=False)
    stream1 = dp("stream1", [128, T1, FW1], BF16, isOutput=False)
    wa_p = dp("wa_b", [128, D], BF16, isOutput=False)
    wt_p = dp("wt", [D, D], BF16, isOutput=False)
    bt_p = dp("bt", [128, 1], F32, isOutput=False)
    wih_p = dp("wih", [D, 3 * D], BF16, isOutput=False)
    whh_p = dp("whh", [D, 3 * D], BF16, isOutput=False)
    biasr_p = dp("biasr", [128, 1], F32, isOutput=False)
    biasz_p = dp("biasz", [128, 1], F32, isOutput=False)
    nbiasz_p = dp("nbiasz", [128, 1], F32, isOutput=False)
    biasin_p = dp("biasin", [128, 1], F32, isOutput=False)
    biashn_p = dp("biashn", [128, 1], F32, isOutput=False)
    w1_p = dp("w1", [D, D], BF16, isOutput=False)
    b1_p = dp("b1", [128, 1], F32, isOutput=False)
    w2_p = dp("w2", [D, D], BF16, isOutput=False)
    b2_p = dp("b2", [128, 1], F32, isOutput=False)
    w3_p = dp("w3", [128, 1], F32, isOutput=False)
    wtr0_p = dp("wtr0", [D, D], BF16, isOutput=False)
    wtr1_p = dp("wtr1", [D, D], BF16, isOutput=False)
    btr_p = dp("btr", [128, 1], F32, isOutput=False)
    bmask_p = dp("bmask", [128, T1 * BC], BF16, isOutput=False)
    ght_o = dp("ghtT", [128, BC], F32, isOutput=True)

    with tile.TileContext(nc) as tc:
        with (
            tc.tile_pool(name="const", bufs=1) as const,
            tc.tile_pool(name="stream", bufs=3) as stream,
            tc.tile_pool(name="work", bufs=2) as work,
        ):
            ident = const.tile([128, 128], F32)
            make_identity(nc, ident[:])
            identb = const.tile([128, 128], BF16)
            nc.gpsimd.memset(identb[:], 0.0)
            nc.vector.tensor_scalar_add(out=identb[:], in0=ident[:], scalar1=0.0)

            def load(name, p, shape, dt):
                t = const.tile(shape, dt, tag=name)
                nc.sync.dma_start(out=t[:], in_=p[:])
                return t

            wa_b = load("wa_b", wa_p, [128, D], BF16)
            wt_t = load("wt", wt_p, [D, D], BF16)
            bt_c = load("bt", bt_p, [128, 1], F32)
            wih_t = load("wih", wih_p, [D, 3 * D], BF16)
            whh_t = load("whh", whh_p, [D, 3 * D], BF16)
            small = {}
            for nm, p in (("biasr", biasr_p), ("biasz", biasz_p),
                          ("nbiasz", nbiasz_p), ("biasin", biasin_p),
                          ("biashn", biashn_p), ("b1", b1_p), ("b2", b2_p),
                          ("w3", w3_p), ("btr", btr_p)):
                small[nm] = load(nm, p, [128, 1], F32)
            w1_t = load("w1", w1_p, [D, D], BF16)
            w2_t = load("w2", w2_p, [D, D], BF16)
            wtr0_t = load("wtr0", wtr0_p, [D, D], BF16)
            wtr1_t = load("wtr1", wtr1_p, [D, D], BF16)
            bm_t = load("bmask", bmask_p, [128, T1 * BC], BF16)
            ones_c = const.tile([128, 1], BF16)
            nc.gpsimd.memset(ones_c[:], 1.0)
            zero32 = const.tile([128, BC], BF16)
            nc.gpsimd.memset(zero32[:], 0.0)

            neib0 = const.tile([128, T0 * D], BF16)  # hop-0 out, token-major
            xt = const.tile([128, F1P], BF16)  # hop-1 out, D-major
            outt = const.tile([128, F1P], BF16)  # GRU out, D-major
            nc.gpsimd.memset(outt[:, F1:], 0.0)
            w3s = const.tile([128, F1P], BF16)
            albuf = const.tile([128, T1], F32)

            hpsum = tc.tile_pool(name="hpsum", bufs=1, space="PSUM")
            gpsum = tc.tile_pool(name="gpsum", bufs=1, space="PSUM")
            psum = hpsum.__enter__()
            gps_pool = gpsum.__enter__()
            pools = (stream, work, psum)
            G = 4

            def hop0_group(j):
                st = stream.tile([128, G * FW], BF16, tag="st0")
                st3 = st[:].rearrange("p (u f) -> p u f", u=G)
                nc.sync.dma_start(out=st3, in_=stream0[:, j * G:(j + 1) * G])
                nbv = st3[:, :, D + NB * D:FW]

                def out0(mm, j=j):
                    nbt = work.tile([128, G * D], BF16, tag="nbt")
                    nc.scalar.add(out=nbt[:], in_=mm[:], add=bt_c[:, :1])
                    for u in range(G):
                        tb = psum.tile([128, D], BF16, tag="tb")
                        nc.tensor.transpose(out=tb[:],
                                            in_=nbt[:, u * D:(u + 1) * D],
                                            identity=identb[:])
                        g = (j * G + u) * D
                        nc.scalar.copy(out=neib0[:, g:g + D], in_=tb[:])

                _hop_group(nc, pools, G, st3, nbv, wa_b, wt_t, bt_c,
                           identb, out0)

            def hop1_group(lo, G1):
                st = stream.tile([128, 4 * FW1], BF16, tag="st1")
                st3 = st[:].rearrange("p (u f) -> p u f", u=4)[:, :G1]
                nc.sync.dma_start(out=st3, in_=stream1[:, lo:lo + G1])
                nbv = neib0[:, lo * NB * D:(lo + G1) * NB * D].rearrange(
                    "p (u f) -> p u f", u=G1)

                def out1(mm, lo=lo, G1=G1):
                    nc.scalar.add(out=xt[:, lo * D:(lo + G1) * D],
                                  in_=mm[:], add=bt_c[:, :1])

                _hop_group(nc, pools, G1, st3, nbv, wa_b, wt_t, bt_c,
                           identb, out1)

            def gru_step(t):
                x_t = xt[:, t * BC:(t + 1) * BC]
                h_prev = (outt[:, (t - 1) * BC:t * BC] if t > 0 else zero32[:])
                psr = gps_pool.tile([128, BC], F32, tag="psr")
                nc.tensor.matmul(out=psr[:], lhsT=wih_t[:, 0:D], rhs=x_t,
                                 start=True, stop=False)
                nc.tensor.matmul(out=psr[:], lhsT=whh_t[:, 0:D], rhs=h_prev,
                                 start=False, stop=True)
                r = work.tile([128, BC], BF16, tag="gr")
                nc.scalar.activation(out=r[:], in_=psr[:], func=AF.Sigmoid,
                                     bias=small["biasr"][:, :1])
                psz = gps_pool.tile([128, BC], F32, tag="psz")
                nc.tensor.matmul(out=psz[:], lhsT=wih_t[:, D:2 * D], rhs=x_t,
                                 start=True, stop=False)
                nc.tensor.matmul(out=psz[:], lhsT=whh_t[:, D:2 * D], rhs=h_prev,
                                 start=False, stop=True)
                z = work.tile([128, BC], BF16, tag="gz")
                nc.scalar.activation(out=z[:], in_=psz[:], func=AF.Sigmoid,
                                     bias=small["biasz"][:, :1])
                zm = work.tile([128, BC], BF16, tag="gzm")
                nc.scalar.activation(out=zm[:], in_=psz[:], func=AF.Sigmoid,
                                     scale=-1.0, bias=small["nbiasz"][:, :1])
                zh = work.tile([128, BC], BF16, tag="zh")
                nc.vector.tensor_tensor(out=zh[:], in0=z[:], in1=h_prev,
                                        op=ALU.mult)
                psin = gps_pool.tile([128, BC], F32, tag="psin")
                nc.tensor.matmul(out=psin[:], lhsT=wih_t[:, 2 * D:], rhs=x_t,
                                 start=True, stop=True)
                pshn = gps_pool.tile([128, BC], F32, tag="pshn")
                nc.tensor.matmul(out=pshn[:], lhsT=whh_t[:, 2 * D:], rhs=h_prev,
                                 start=True, stop=True)
                psinb = work.tile([128, BC], BF16, tag="psinb")
                nc.vector.tensor_scalar_add(out=psinb[:], in0=psin[:],
                                            scalar1=small["biasin"][:, :1])
                rhn = work.tile([128, BC], BF16, tag="rhn")
                nc.vector.scalar_tensor_tensor(
                    out=rhn[:], in0=pshn[:], scalar=small["biashn"][:, :1],
                    in1=r[:], op0=ALU.add, op1=ALU.mult)
                pre = work.tile([128, BC], BF16, tag="pre")
                nc.vector.tensor_tensor(out=pre[:], in0=rhn[:], in1=psinb[:],
                                        op=ALU.add)
                n_sb = work.tile([128, BC], BF16, tag="n")
                nc.scalar.activation(out=n_sb[:], in_=pre[:], func=AF.Tanh)
                nzm = work.tile([128, BC], BF16, tag="nzm")
                nc.vector.tensor_tensor(out=nzm[:], in0=n_sb[:], in1=zm[:],
                                        op=ALU.mult)
                nc.vector.tensor_tensor(out=outt[:, t * BC:(t + 1) * BC],
                                        in0=nzm[:], in1=zh[:], op=ALU.add)

            # interleave: hop-0 groups feed hop-1 groups feed GRU steps so the
            # GRU dependency chain hides under hop DVE work
            done0 = 0
            for lo, G1 in ((0, 4), (4, 4), (8, 4), (12, 1)):
                need0 = 2 * (lo + G1)
                while done0 < need0:
                    hop0_group(done0)
                    done0 += 1
                hop1_group(lo, G1)
                for t in range(lo * 4, min(S, (lo + G1) * 4)):
                    gru_step(t)
            gpsum.__exit__(None, None, None)
            hpsum.__exit__(None, None, None)

            # ---- attention pooling ----
            ppsum = tc.tile_pool(name="ppsum", bufs=1, space="PSUM")
            psum = ppsum.__enter__()
            loc = work.tile([128, BC], BF16, tag="loc")
            nc.scalar.copy(out=loc[:], in_=outt[:, (S - 1) * BC:S * BC])
            q1ps = psum.tile([128, BC], F32, tag="q1")
            nc.tensor.matmul(out=q1ps[:], lhsT=w1_t[:], rhs=loc[:], start=True,
                             stop=True)
            q1s = work.tile([128, BC], BF16, tag="q1s")
            nc.scalar.add(out=q1s[:], in_=q1ps[:], add=small["b1"][:, :1])
            CH = 10 * BC  # 320 cols = 10 s-steps per chunk (s-major layout)
            for k in range(5):
                ps = psum.tile([128, CH], F32, tag="q2")
                nc.tensor.matmul(out=ps[:], lhsT=w2_t[:],
                                 rhs=outt[:, k * CH:(k + 1) * CH],
                                 start=True, stop=True)
                v = work.tile([128, CH], BF16, tag="v")
                q1b = q1s[:].unsqueeze(1).broadcast_to([128, 10, BC])
                vv = v[:].rearrange("p (s b) -> p s b", b=BC)
                psv = ps[:].rearrange("p (s b) -> p s b", b=BC)
                nc.vector.tensor_tensor(out=vv, in0=psv, in1=q1b, op=ALU.add)
                sg = work.tile([128, CH], BF16, tag="sg")
                nc.scalar.activation(out=sg[:], in_=v[:], func=AF.Sigmoid,
                                     bias=small["b2"][:, :1])
                nc.vector.tensor_scalar_mul(out=w3s[:, k * CH:(k + 1) * CH],
                                            in0=sg[:], scalar1=small["w3"][:, :1])
            alps = psum.tile([128, T1], F32, tag="alps")
            for i in range(T1):
                w = min(128, F1 - i * 128)
                if w <= 0:
                    break
                nc.tensor.matmul(out=alps[:w, i:i + 1],
                                 lhsT=w3s[:, i * 128:i * 128 + w], rhs=ones_c[:],
                                 start=True, stop=True)
            nc.scalar.copy(out=albuf[:], in_=alps[:])
            gps = psum.tile([BC, 128], F32, tag="gps")
            for i in range(T1):
                mt = work.tile([128, BC], BF16, tag="mt")
                nc.vector.tensor_scalar_mul(
                    out=mt[:], in0=bm_t[:, i * BC:(i + 1) * BC],
                    scalar1=albuf[:, i:i + 1])
                tpp = psum.tile([128, 128], BF16, tag="tpp")
                nc.tensor.transpose(out=tpp[:],
                                    in_=outt[:, i * 128:(i + 1) * 128],
                                    identity=identb[:])
                onat = work.tile([128, 128], BF16, tag="onat")
                nc.scalar.copy(out=onat[:], in_=tpp[:])
                nc.tensor.matmul(out=gps[:], lhsT=mt[:], rhs=onat[:],
                                 start=(i == 0), stop=(i == T1 - 1))
            gsb = work.tile([BC, 128], BF16, tag="gsb")
            nc.scalar.copy(out=gsb[:], in_=gps[:])
            gtp = psum.tile([128, BC], BF16, tag="gtp")
            nc.tensor.transpose(out=gtp[:], in_=gsb[:],
                                identity=identb[:BC, :BC])
            g_t = work.tile([128, BC], BF16, tag="g_t")
            nc.scalar.copy(out=g_t[:], in_=gtp[:])
            ghps = psum.tile([128, BC], F32, tag="ghps")
            nc.tensor.matmul(out=ghps[:], lhsT=wtr0_t[:], rhs=loc[:], start=True,
                             stop=False)
            nc.tensor.matmul(out=ghps[:], lhsT=wtr1_t[:], rhs=g_t[:], start=False,
                             stop=True)
            ghsb = work.tile([128, BC], F32, tag="ghsb")
            nc.scalar.add(out=ghsb[:], in_=ghps[:], add=small["btr"][:, :1])
            nc.sync.dma_start(out=ght_o[:], in_=ghsb[:])
            ppsum.__exit__(None, None, None)
    nc.compile()
    return nc


def build_launch_b():
    nc = bacc.Bacc(None)
    dp = nc.declare_dram_parameter
    ghtT = dp("ghtT", [128, B], BF16, isOutput=False)
    itemT = dp("itemT", [128, VSP], BF16, isOutput=False)
    out = dp("logits", [B, VSP], BF16, isOutput=True)
    with tile.TileContext(nc) as tc:
        with (
            tc.tile_pool(name="const", bufs=1) as const,
            tc.tile_pool(name="stream", bufs=4) as stream,
            tc.tile_pool(name="work", bufs=4) as work,
            tc.tile_pool(name="psum", bufs=4, space="PSUM") as psum,
        ):
            gh = const.tile([128, B], BF16)
            nc.sync.dma_start(out=gh[:], in_=ghtT[:])
            ob0 = const.tile([128, VSP], BF16, tag="ob0")
            ob1 = const.tile([128, VSP], BF16, tag="ob1")
            ob = [ob0, ob1]
            CW = 2560  # 5 psum-chunks per stream tile
            for g in range(VSP // CW):
                it = stream.tile([128, CW], BF16, tag="it")
                nc.sync.dma_start(out=it[:], in_=itemT[:, g * CW:(g + 1) * CW])
                for k in range(CW // 512):
                    c = g * (CW // 512) + k
                    for bh in range(2):
                        ps = psum.tile([128, 512], F32, tag="ps")
                        nc.tensor.matmul(out=ps[:],
                                         lhsT=gh[:, bh * 128:(bh + 1) * 128],
                                         rhs=it[:, k * 512:(k + 1) * 512],
                                         start=True, stop=True)
                        dst = ob[bh][:, c * 512:(c + 1) * 512]
                        if (c + bh) % 2 == 0:
                            nc.scalar.activation(out=dst, in_=ps[:],
                                                 func=AF.Relu)
                        else:
                            nc.vector.tensor_scalar_max(out=dst, in0=ps[:],
                                                        scalar1=0.0)
                # flush finished 2560-wide slabs so the store overlaps compute
                for bh in range(2):
                    nc.sync.dma_start(
                        out=out[bh * 128:(bh + 1) * 128, g * CW:(g + 1) * CW],
                        in_=ob[bh][:, g * CW:(g + 1) * CW])

    nc.compile()
    return nc


def _prep_core(c, h_iids, adj_entity, adj_relation, item_emb, rel_emb):
    h = h_iids[c * BC:(c + 1) * BC].astype(np.int64)  # [32, 50]
    hsm = np.ascontiguousarray(h.T).reshape(-1)  # s-major: tok = s*BC + b
    e1 = adj_entity[hsm].reshape(-1)  # [12800] flat, s-major tokens
    e2 = adj_entity[e1].reshape(-1, NB)  # [12800, 8]
    r1 = adj_relation[e1].reshape(-1, NB)  # [12800, 8]
    r0 = adj_relation[hsm].reshape(-1, NB)  # [1600, 8]

    npad0 = T0 * 128  # 13312
    e1p = np.zeros(npad0, np.int64)
    e1p[:BC * S * NB] = e1
    e2p = np.zeros((npad0, NB), np.int64)
    e2p[:e2.shape[0]] = e2
    r1p = np.zeros((npad0, NB), np.int64)
    r1p[:r1.shape[0]] = r1

    u = np.arange(T0)
    p = np.arange(128)
    perm = 1024 * (u[:, None] // 8) + 8 * p[None, :] + (u[:, None] % 8)

    self0 = item_emb[e1p[perm]]  # [104,128,128]
    nb0 = item_emb[e2p[perm]].reshape(T0, 128, NB * D)
    rel0 = rel_emb[r1p[perm]].reshape(T0, 128, NB * D)
    stream0 = np.concatenate([self0, rel0, nb0], axis=2)  # [104,128,2176]
    stream0 = np.ascontiguousarray(
        stream0.transpose(1, 0, 2)).astype(ml_dtypes.bfloat16)

    hf = np.zeros(F1P, np.int64)
    hf[:F1] = hsm
    r0p = np.zeros((F1P, NB), np.int64)
    r0p[:F1] = r0
    self1 = item_emb[hf].reshape(T1, 128, D)
    rel1 = rel_emb[r0p].reshape(T1, 128, NB * D)
    stream1 = np.concatenate([self1, rel1], axis=2)  # [13,128,1152]
    stream1 = np.ascontiguousarray(
        stream1.transpose(1, 0, 2)).astype(ml_dtypes.bfloat16)

    fl = np.arange(F1P)
    bmask = np.zeros((F1P, BC), np.float32)
    valid = fl < F1
    bmask[valid, fl[valid] % BC] = 1.0
    bmask = np.ascontiguousarray(
        bmask.reshape(T1, 128, BC).transpose(1, 0, 2).reshape(128, T1 * BC)
    ).astype(ml_dtypes.bfloat16)
    return dict(stream0=stream0, stream1=stream1, bmask=bmask)


def kernel(h_iids, a_iids, adj_entity, adj_relation, item_emb, rel_emb,
           Wa, ba, Wt, bt, Wih, Whh, bih, bhh,
           W1, b1, W2, b2, W3, Wtr, btr):
    h_iids = np.asarray(h_iids)
    adj_entity = np.asarray(adj_entity)
    adj_relation = np.asarray(adj_relation)
    item_emb = np.asarray(item_emb, np.float32)
    rel_emb = np.asarray(rel_emb, np.float32)

    if "a" not in _CACHE:
        _CACHE["a"] = build_launch_a()
    if "b" not in _CACHE:
        _CACHE["b"] = build_launch_b()
    nc_a, nc_b = _CACHE["a"], _CACHE["b"]

    bf = lambda x: np.ascontiguousarray(np.asarray(x, np.float32)).astype(
        ml_dtypes.bfloat16)
    col = lambda x: np.ascontiguousarray(np.asarray(x, np.float32).reshape(-1, 1))
    bihf = np.asarray(bih, np.float32)
    bhhf = np.asarray(bhh, np.float32)
    # ba shifts all pre-softmax scores equally within each softmax group, so it
    # cancels; it is intentionally unused.
    weights = dict(
        wa_b=bf(np.broadcast_to(np.asarray(Wa, np.float32).reshape(1, D),
                                (128, D))),
        wt=bf(Wt), bt=col(bt),
        wih=bf(Wih), whh=bf(Whh),
        biasr=col(bihf[:D] + bhhf[:D]),
        biasz=col(bihf[D:2 * D] + bhhf[D:2 * D]),
        nbiasz=col(-(bihf[D:2 * D] + bhhf[D:2 * D])),
        biasin=col(bihf[2 * D:]),
        biashn=col(bhhf[2 * D:]),
        w1=bf(W1), b1=col(b1), w2=bf(W2), b2=col(b2), w3=col(W3),
        wtr0=bf(np.asarray(Wtr, np.float32)[:D]),
        wtr1=bf(np.asarray(Wtr, np.float32)[D:]),
        btr=col(btr),
    )
    in_maps = []
    for c in range(NCORE):
        m = _prep_core(c, h_iids, adj_entity, adj_relation, item_emb, rel_emb)
        m.update(weights)
        in_maps.append(m)
    res_a = _run(nc_a, in_maps, "A")
    ghtT = np.concatenate([res_a.results[c]["ghtT"] for c in range(NCORE)],
                          axis=1).astype(np.float32)  # [128, 256]

    itemT_full = np.ascontiguousarray(item_emb.T)  # [128, 100000]
    ght_b = np.ascontiguousarray(ghtT).astype(ml_dtypes.bfloat16)
    in_maps_b = []
    for c in range(NCORE):
        sl = np.zeros((128, VSP), np.float32)
        sl[:, :VS] = itemT_full[:, c * VS:(c + 1) * VS]
        in_maps_b.append({"ghtT": ght_b, "itemT": sl.astype(ml_dtypes.bfloat16)})
    res_b = _run(nc_b, in_maps_b, "B")
    logits = np.concatenate(
        [np.asarray(res_b.results[c]["logits"][:, :VS], np.float32)
         for c in range(NCORE)], axis=1)
    return logits
